# revision 37
# baseline (speedup 1.0000x reference)
"""Trainium2 Bass kernel for sparse (bi-level routing) attention.

Contract: kernel(**inputs) takes the FULL unsharded inputs (as produced by
setup_inputs) and returns the FULL output (64, 1536), float32.

Strategy (pure data-parallel over batch, 8 samples per NeuronCore):
  - Host: reshape x into region-major channel-major layout (x_rm) and its
    transpose (xT_rm), pre-transpose weights, replicate them to all cores.
  - Device, per core:
      q       = Wq @ x_cls + bq                        (exact fp32 matmul)
      qkT     = Wk.T @ blockdiag(scale * q)            (per-head q @ Wk_m)
      k       = Wk @ x_rm          (fp32r matmuls, region-major free dim)
      k_r     = maxpool over the 16 positions of each of the 64 regions
      a_r     = q . k_r            (exact fp32 matmul)
      top-32 regions via vector.max8 + match_replace; additive -1e30 mask
      L       = qkT_s.T @ x_rm     (12, 1024) logits for every position
      p       = softmax(L + mask)  (zeros outside routed regions)
      pT      = PE transpose of p
      xw      = pT.T @ xT_rm       (12, 768)  softmax-weighted input rows
      o_full  = Wv @ xw.T          (768, 12); take per-head diagonal + bv
      out     = o_diag.T @ Wo.T + bo
  Dropping bkv[:C] is exact: it shifts a_r by a per-sample constant and the
  logits by a per-(sample, head) constant, neither of which changes top-k
  or softmax. bkv[C:] enters as +bv because softmax weights sum to 1.

This file is self-contained: shapes/sharding are hardcoded.
"""

import sys
import numpy as np

sys.path.insert(0, "/opt/trn_rl_repo")

import concourse.bass as bass
import concourse.bacc as bacc
import concourse.mybir as mybir
from concourse import tile
from concourse.bass_utils import run_bass_kernel_spmd

F32 = mybir.dt.float32
F32R = mybir.dt.float32r
BF16 = mybir.dt.bfloat16
AF = mybir.ActivationFunctionType
ALU = mybir.AluOpType
AX = mybir.AxisListType

NCORES = 8
B, N, C = 64, 1025, 768
PC = B // NCORES          # samples per core = 8
NH, HD = 12, 64           # heads, head dim
R, S = 64, 16             # regions, tokens per region
TOPK = 32
NCH = C // 128            # 6 channel chunks
SCALE = float(C) ** -0.5
NEG = -1.0e30


def r32(ap):
    return ap.bitcast(F32R)


def build_nc(n_samples=PC, exact_k=False, repeat=1, debug_taps=False):
    GSD = max(1, n_samples // 2)
    """Build the per-core Bass program. Same program runs on all 8 cores."""
    nc = bacc.Bacc("TRN2", target_bir_lowering=False, debug=False)

    # ---- DRAM I/O ----
    d_xct = nc.dram_tensor("xct", [C, n_samples], F32, kind="ExternalInput")
    d_xrm = nc.dram_tensor("xrm", [n_samples, C, R * S], F32, kind="ExternalInput")
    d_xtm = nc.dram_tensor("xtm", [n_samples, R * S, C], F32, kind="ExternalInput")
    d_wqt = nc.dram_tensor("wqt", [C, C], F32, kind="ExternalInput")
    d_wk = nc.dram_tensor("wk", [C, C], F32, kind="ExternalInput")
    d_wkt = nc.dram_tensor("wkt", [C, C], F32, kind="ExternalInput")
    d_wvt = nc.dram_tensor("wvt", [C, C], F32, kind="ExternalInput")
    d_wot = nc.dram_tensor("wot", [C, 2 * C], F32, kind="ExternalInput")
    d_bq = nc.dram_tensor("bq", [C, 1], F32, kind="ExternalInput")
    d_bv = nc.dram_tensor("bv", [C, 1], F32, kind="ExternalInput")
    d_bob = nc.dram_tensor("bob", [n_samples, 2 * C], F32, kind="ExternalInput")
    d_ident = nc.dram_tensor("ident", [128, 128], F32, kind="ExternalInput")
    d_zeros = nc.dram_tensor("zeros", [128, NCH * n_samples * NH], F32R, kind="ExternalInput")
    d_rep12 = nc.dram_tensor("rep12", [max(1, n_samples // 2), max(1, n_samples // 2) * NH], F32, kind="ExternalInput")
    d_out = nc.dram_tensor("out", [n_samples, 2 * C], F32, kind="ExternalOutput")
    if debug_taps:
        d_dbg_ar = nc.dram_tensor("dbg_ar", [n_samples, R], F32, kind="ExternalOutput")
        d_dbg_madd = nc.dram_tensor("dbg_madd", [n_samples, R], F32, kind="ExternalOutput")
        d_dbg_L = nc.dram_tensor("dbg_L", [NH, R * S], F32, kind="ExternalOutput")
        d_dbg_xw = nc.dram_tensor("dbg_xw", [NH, C], F32, kind="ExternalOutput")
        d_dbg_q = nc.dram_tensor("dbg_q", [128, NCH * n_samples], F32, kind="ExternalOutput")
        d_dbg_kr = nc.dram_tensor("dbg_kr", [128, NCH * R], F32, kind="ExternalOutput")
        d_dbg_xt = nc.dram_tensor("dbg_xt", [128, 8 * C], mybir.dt.bfloat16, kind="ExternalOutput")
        d_dbg_xtraw = nc.dram_tensor("dbg_xtraw", [128, 8 * C], mybir.dt.bfloat16, kind="ExternalOutput")
        d_dbg_pt = nc.dram_tensor("dbg_pt", [128, 8 * GSD * NH], mybir.dt.bfloat16, kind="ExternalOutput")

    kd = F32 if exact_k else F32R

    def chunked(dram_ap, nch, width):
        return dram_ap.rearrange("(n p) m -> p n m", p=128)

    with tile.TileContext(nc) as tc:
        with (
            tc.tile_pool(name="wpool", bufs=1) as wpool,
            tc.tile_pool(name="spool", bufs=1) as spool,     # small persistents
            tc.tile_pool(name="xpool", bufs=2) as xpool,     # x_rm per sample
            tc.tile_pool(name="tpool", bufs=1) as tpool,     # xT_rm per sample
            tc.tile_pool(name="mpool", bufs=2) as mpool,     # misc per-sample
            tc.tile_pool(name="ppsum", bufs=2, space="PSUM") as ppsum,
        ):
            # ---------- load persistent weights ----------
            wkt = wpool.tile([128, NCH, C], F32, tag="wkt")
            nc.sync.dma_start(wkt[:], chunked(d_wkt.ap(), NCH, C))
            wvt = wpool.tile([128, NCH, C], F32, tag="wvt")
            nc.sync.dma_start(wvt[:], chunked(d_wvt.ap(), NCH, C))
            wot = wpool.tile([128, NCH, 2 * C], F32, tag="wot")
            nc.sync.dma_start(wot[:], chunked(d_wot.ap(), NCH, 2 * C))
            ident = spool.tile([128, 128], F32, tag="ident")
            nc.sync.dma_start(ident[:], d_ident.ap())
            bq_sb = spool.tile([128, NCH], F32, tag="bq")
            nc.sync.dma_start(bq_sb[:], d_bq.ap().rearrange("(n p) m -> p (n m)", p=128))
            bv_sb = spool.tile([128, NCH], F32, tag="bv")
            nc.sync.dma_start(bv_sb[:], d_bv.ap().rearrange("(n p) m -> p (n m)", p=128))
            bob_sb = spool.tile([n_samples, 2 * C], F32, tag="bob")
            nc.sync.dma_start(bob_sb[:], d_bob.ap())

            # ---------- prologue: q, Qblk, qkT ----------
            # transient weights ride the x/t pools' slots (same shapes)
            wqtt = xpool.tile([128, NCH, C], F32, tag="xs")
            nc.sync.dma_start(wqtt[:], chunked(d_wqt.ap(), NCH, C))
            wknt = tpool.tile([128, NCH, C], F32, tag="xt")
            nc.sync.dma_start(wknt[:], chunked(d_wk.ap(), NCH, C))
            xct = spool.tile([128, NCH, n_samples], F32, tag="xct")
            nc.sync.dma_start(
                xct[:], d_xct.ap().rearrange("(n p) m -> p n m", p=128)
            )

            q_all = spool.tile([128, NCH, n_samples], F32, tag="q")
            for oc in range(NCH):
                ps = ppsum.tile([128, n_samples], F32, tag="acc", bufs=3)
                for cc in range(NCH):
                    nc.tensor.matmul(
                        ps[:],
                        wqtt[:, cc, bass.ts(oc, 128)],
                        xct[:, cc, :],
                        start=(cc == 0),
                        stop=(cc == NCH - 1),
                    )
                # q = psum + bq  (Identity supports AP bias)
                nc.scalar.activation(
                    q_all[:, oc, :], ps[:], AF.Identity,
                    bias=bq_sb[:, oc : oc + 1], scale=1.0,
                )

            qblk = spool.tile([128, NCH, n_samples * NH], F32, tag="qblk")
            nc.vector.memset(qblk[:], 0.0)
            for cc in range(NCH):
                for j in (0, 1):
                    m = 2 * cc + j
                    dst = qblk[:, cc, :].rearrange(
                        "p (s m) -> p s m", m=NH
                    )[j * 64 : j * 64 + 64, :, m]
                    src = q_all[j * 64 : j * 64 + 64, cc, :]
                    nc.scalar.activation(dst, src, AF.Copy, scale=SCALE)

            qkt = spool.tile([128, NCH, n_samples * NH], F32, tag="qkt")
            for oc in range(NCH):
                ps = ppsum.tile([128, n_samples * NH], F32, tag="acc", bufs=3)
                for cc in range(NCH):
                    nc.tensor.matmul(
                        ps[:],
                        r32(wknt[:, cc, bass.ts(oc, 128)]),
                        r32(qblk[:, cc, :]),
                        start=(cc == 0),
                        stop=(cc == NCH - 1),
                    )
                nc.scalar.copy(qkt[:, oc, :], ps[:])

            # xwT accumulated across samples for the epilogue
            xwt = spool.tile([128, NCH, n_samples * NH], F32R, tag="xwt")

            GS = max(1, n_samples // 2)          # samples per pipeline group
            NG = n_samples // GS
            GM = GS * NH

            rep12 = spool.tile([GS, GM], F32, tag="rep12")
            nc.sync.dma_start(rep12[:], d_rep12.ap())

            artmp = spool.tile([1, n_samples, R], F32, tag="artmp")
            mpool_tags = {}
            L48s = {}
            pTs = {}

            def pass1_sample(s):
                xs = xpool.tile([128, NCH, R * S], F32R, tag="xs", name=f"xs{s}")
                nc.sync.dma_start(xs[:], chunked(d_xrm.ap()[s], NCH, R * S))
                g = s // GS
                if g not in L48s:
                    L48s[g] = spool.tile([GM, R * S], F32, tag="L48", bufs=2,
                                         name=f"L48_{g}")
                kr = mpool.tile([128, NCH, R], F32, tag="kr", name=f"kr{s}")
                for oc in range(NCH):
                    for half in (0, 1):
                        hsl = bass.ts(half, 512)
                        ps = ppsum.tile([128, 512], F32, tag="acc", bufs=3,
                                        name=f"pk{s}_{oc}_{half}")
                        for cc in range(NCH):
                            nc.tensor.matmul(
                                ps[:],
                                wkt[:, cc, bass.ts(oc, 128)],
                                xs[:, cc, hsl],
                                start=(cc == 0),
                                stop=(cc == NCH - 1),
                            )
                        # region max-pool straight out of PSUM
                        nc.vector.tensor_reduce(
                            kr[:, oc, bass.ts(half, 32)],
                            ps[:].rearrange("p (r s) -> p r s", s=S),
                            axis=AX.X,
                            op=ALU.max,
                        )
                psl = ppsum.tile([NH, R * S], F32, tag="pl", bufs=1, name=f"pL{s}")
                for half in (0, 1):
                    hsl = bass.ts(half, 512)
                    for cc in range(NCH):
                        nc.tensor.matmul(
                            psl[:, hsl],
                            qkt[:, cc, bass.ts(s, NH)],
                            xs[:, cc, hsl],
                            start=(cc == 0),
                            stop=(cc == NCH - 1),
                        )
                Ltmp = mpool.tile([NH, R * S], F32, tag="Ltmp", name=f"Lt{s}")
                nc.scalar.copy(Ltmp[:], psl[:])
                nc.gpsimd.dma_start(L48s[g][bass.ts(s - g * GS, NH), :], Ltmp[:])

                # routing scores a_r = q . k_r  (exact fp32)
                psa = ppsum.tile([1, R], F32, tag="pa", bufs=1, name=f"pa{s}")
                for oc in range(NCH):
                    nc.tensor.matmul(
                        psa[:],
                        q_all[:, oc, s : s + 1],
                        kr[:, oc, :],
                        start=(oc == 0),
                        stop=(oc == NCH - 1),
                    )
                nc.scalar.copy(artmp[0:1, s, :], psa[:])
                if debug_taps and s == 0:
                    nc.sync.dma_start(
                        d_dbg_kr.ap(), kr[:].rearrange("p a b -> p (a b)"))
                    nc.sync.dma_start(
                        d_dbg_q.ap(), q_all[:].rearrange("p a b -> p (a b)"))

            def batch_group(g):
                L48 = L48s[g]
                ar_g = mpool.tile([GS, R], F32, tag="ar_g", bufs=2, name=f"ar{g}")
                if g == 0:
                    mpool_tags["ar_g0"] = ar_g
                nc.gpsimd.dma_start(ar_g[:], artmp[0:1, g * GS : (g + 1) * GS, :])

                wkA = mpool.tile([GS, R], F32, tag="wkA", bufs=2, name=f"wA{g}")
                wkB = mpool.tile([GS, R], F32, tag="wkB", bufs=2, name=f"wB{g}")
                m8 = mpool.tile([GS, 8], F32, tag="m8", bufs=2, name=f"m8{g}")
                nc.vector.tensor_copy(wkA[:], ar_g[:])
                cur, nxt = wkA, wkB
                for _ in range(4):
                    nc.vector.max(m8[:], cur[:])
                    nc.vector.match_replace(nxt[:], m8[:], cur[:], NEG)
                    cur, nxt = nxt, cur
                madd = mpool.tile([GS, R], F32, tag="madd", bufs=2, name=f"md{g}")
                if g == 0:
                    mpool_tags["madd0"] = madd
                nc.vector.tensor_tensor(madd[:], ar_g[:], cur[:], ALU.is_gt)
                nc.vector.tensor_scalar(
                    madd[:], madd[:], 1.0e30, 1.0e30, ALU.mult, ALU.subtract
                )
                # replicate each sample's region mask to its 12 head rows
                psb = ppsum.tile([GM, R], F32, tag="pa", bufs=1, name=f"pb{g}")
                nc.tensor.matmul(psb[:], rep12[:], madd[:], start=True, stop=True)
                madd48 = mpool.tile([GM, R], F32, tag="madd48", bufs=2, name=f"m4{g}")
                nc.scalar.copy(madd48[:], psb[:])

                # masked softmax over all positions (no max-subtraction:
                # logits are O(1) for this problem's data distribution)
                nc.vector.tensor_tensor(
                    L48[:].rearrange("p (r s) -> p r s", s=S),
                    L48[:].rearrange("p (r s) -> p r s", s=S),
                    madd48[:].unsqueeze(-1).broadcast_to((GM, R, S)),
                    ALU.add,
                )
                Zt = mpool.tile([GM, 1], F32, tag="Z", bufs=2, name=f"Z{g}")
                nc.scalar.activation(L48[:], L48[:], AF.Exp, accum_out=Zt[:])
                Rt = mpool.tile([GM, 1], F32, tag="Rt", bufs=2, name=f"R{g}")
                nc.vector.reciprocal(Rt[:], Zt[:])
                nc.vector.tensor_scalar_mul(L48[:], L48[:], Rt[:])

                # transpose this group's p: 8 chunks of (GM,128)->(128,GM)
                pT = mpool.tile([128, 8, GM], F32R, tag="pT", bufs=2, name=f"pT{g}")
                pTs[g] = pT
                for pc in range(8):
                    pst = ppsum.tile([128, GM], F32, tag="pt", bufs=2,
                                     name=f"pp{g}_{pc}")
                    nc.tensor.transpose(
                        pst[:], L48[:, bass.ts(pc, 128)], ident[:GM, :GM]
                    )
                    nc.scalar.copy(pT[:, pc, :], pst[:])
                debug_tap_group(g)

            xw48s = {}

            def pass2_sample(s):
                g = s // GS
                if g not in xw48s:
                    xw48s[g] = spool.tile([GM, C], F32, tag="xw48", bufs=2,
                                          name=f"xw48_{g}")
                pT = pTs[g]
                # two half tiles, double-buffered in the same footprint:
                # pc 0-3 stream from xta while xtb loads, and vice versa
                xta = tpool.tile([128, 4, C], F32R, tag="xt", bufs=2, name=f"xta{s}")
                nc.sync.dma_start(
                    xta[:], d_xtm.ap()[s][0:512, :].rearrange("(n p) m -> p n m", p=128))
                xtb = tpool.tile([128, 4, C], F32R, tag="xt", bufs=2, name=f"xtb{s}")
                nc.sync.dma_start(
                    xtb[:], d_xtm.ap()[s][512:1024, :].rearrange("(n p) m -> p n m", p=128))
                psx = ppsum.tile([NH, C], F32, tag="pl", bufs=1, name=f"px{s}")
                for pc in range(8):
                    half = xta if pc < 4 else xtb
                    for j0, jw in ((0, 512), (512, 256)):
                        nc.tensor.matmul(
                            psx[:, j0 : j0 + jw],
                            pT[:, pc, bass.ts(s - g * GS, NH)],
                            half[:, pc % 4, j0 : j0 + jw],
                            start=(pc == 0),
                            stop=(pc == 7),
                        )
                if debug_taps and s == 0:
                    nc.sync.dma_start(d_dbg_xt.ap(),
                                      xt[:].rearrange("p a b -> p (a b)"))
                    nc.sync.dma_start(d_dbg_pt.ap(),
                                      pT[:].rearrange("p a b -> p (a b)"))
                xwtmp = mpool.tile([NH, C], F32, tag="xwtmp", name=f"xm{s}")
                nc.scalar.copy(xwtmp[:], psx[:])
                nc.gpsimd.dma_start(xw48s[g][bass.ts(s - g * GS, NH), :], xwtmp[:])

            def debug_tap_group(g):
                if not debug_taps or g != 0:
                    return
                nc.sync.dma_start(d_dbg_ar.ap()[0 : GS, :],
                                  mpool_tags["ar_g0"][:])
                nc.sync.dma_start(d_dbg_madd.ap()[0 : GS, :],
                                  mpool_tags["madd0"][:])
                nc.sync.dma_start(d_dbg_L.ap(), L48s[0][0:NH, :])

            def finish_group(g):
                if debug_taps and g == 0:
                    nc.sync.dma_start(d_dbg_xw.ap(), xw48s[0][0:NH, :])
                # transpose xw into xwt chunk columns: 6 of (GM,128)->(128,GM)
                for ccc in range(NCH):
                    pst = ppsum.tile([128, GM], F32, tag="pt", bufs=2,
                                     name=f"pw{g}_{ccc}")
                    nc.tensor.transpose(
                        pst[:], xw48s[g][:, bass.ts(ccc, 128)], ident[:GM, :GM]
                    )
                    nc.scalar.copy(xwt[:, ccc, bass.ts(g, GM)], pst[:])

            # software pipeline: pass1(g+1) interleaves with batch/pass2(g)
            for rep in range(repeat):
                L48s.clear(); pTs.clear(); xw48s.clear()
                for s in range(GS):
                    pass1_sample(s)
                for g in range(NG):
                    nxt = g + 1 < NG
                    if nxt:
                        pass1_sample((g + 1) * GS)
                    batch_group(g)
                    for i in range(GS):
                        if nxt and i >= 1:
                            pass1_sample((g + 1) * GS + i)
                        pass2_sample(g * GS + i)
                    finish_group(g)

            # ---------- epilogue ----------
            od = spool.tile([128, NCH, n_samples], F32, tag="od")
            for oc in range(NCH):
                ps = ppsum.tile([128, n_samples * NH], F32, tag="acc", bufs=3)
                for cc in range(NCH):
                    nc.tensor.matmul(
                        ps[:],
                        r32(wvt[:, cc, bass.ts(oc, 128)]),
                        r32(xwt[:, cc, :]),
                        start=(cc == 0),
                        stop=(cc == NCH - 1),
                    )
                # extract per-head diagonal strips + bv
                for j in (0, 1):
                    m = 2 * oc + j
                    src = ps[:].rearrange("p (s m) -> p s m", m=NH)[
                        j * 64 : j * 64 + 64, :, m
                    ]
                    nc.scalar.activation(
                        od[j * 64 : j * 64 + 64, oc, :], src, AF.Identity,
                        bias=bv_sb[j * 64 : j * 64 + 64, oc : oc + 1],
                    )

            outsb = spool.tile([n_samples, 2 * C], F32, tag="outsb")
            for jc in range(3):
                pso = ppsum.tile([n_samples, 512], F32, tag="acc", bufs=3)
                for cc in range(NCH):
                    nc.tensor.matmul(
                        pso[:],
                        r32(od[:, cc, :]),
                        r32(wot[:, cc, bass.ts(jc, 512)]),
                        start=(cc == 0),
                        stop=(cc == NCH - 1),
                    )
                nc.vector.tensor_tensor(
                    outsb[:, bass.ts(jc, 512)], pso[:],
                    bob_sb[:, bass.ts(jc, 512)], ALU.add,
                )
            nc.sync.dma_start(d_out.ap(), outsb[:])

    nc.compile()
    return nc


def host_prep(x, Wq, bq, Wkv, bkv, Wo, bo, n_samples=PC, ncores=NCORES):
    """Build per-core in_maps."""
    x = np.ascontiguousarray(np.asarray(x, np.float32))
    Wq = np.asarray(Wq, np.float32)
    Wkv = np.asarray(Wkv, np.float32)
    Wo = np.asarray(Wo, np.float32)
    bq = np.asarray(bq, np.float32)
    bkv = np.asarray(bkv, np.float32)
    bo = np.asarray(bo, np.float32)

    Wk, Wv = Wkv[:C], Wkv[C:]
    bv = bkv[C:]
    shared = {
        "wqt": np.ascontiguousarray(Wq.T),
        "wk": np.ascontiguousarray(Wk),
        "wkt": np.ascontiguousarray(Wk.T),
        "wvt": np.ascontiguousarray(Wv.T),
        "wot": np.ascontiguousarray(Wo.T),
        "bq": np.ascontiguousarray(bq[:, None]),
        "bv": np.ascontiguousarray(bv[:, None]),
        "bob": np.ascontiguousarray(np.tile(bo, (n_samples, 1))),
        "ident": np.eye(128, dtype=np.float32),
        "zeros": np.zeros((128, NCH * n_samples * NH), np.float32),
        "rep12": np.kron(np.eye(max(1, n_samples // 2), dtype=np.float32),
                         np.ones((1, NH), np.float32)),
    }

    nb = n_samples * ncores
    xc = x[:nb, 0, :]                                       # (nb, C)
    flat = x[:nb, 1:, :].reshape(nb, -1)
    xs = flat.reshape(nb, C, 8, 4, 8, 4)
    x_rm = np.ascontiguousarray(
        xs.transpose(0, 1, 2, 4, 3, 5).reshape(nb, C, R * S)
    )
    xT_rm = np.ascontiguousarray(x_rm.transpose(0, 2, 1))

    in_maps = []
    for i in range(ncores):
        sl = slice(i * n_samples, (i + 1) * n_samples)
        m = dict(shared)
        m["xct"] = np.ascontiguousarray(xc[sl].T)
        m["xrm"] = x_rm[sl]
        m["xtm"] = xT_rm[sl]
        in_maps.append(m)
    return in_maps


_NC_CACHE = {}


def get_nc(n_samples=PC, exact_k=False, repeat=1):
    key = (n_samples, exact_k, repeat)
    if key not in _NC_CACHE:
        _NC_CACHE[key] = build_nc(n_samples, exact_k, repeat)
    return _NC_CACHE[key]


def kernel(x, Wq, bq, Wkv, bkv, Wo, bo, trace=False):
    if trace:
        try:
            from antenv.axon_hooks import get_axon_ntff_profile_hook  # noqa: F401
        except ImportError:
            trace = False
    in_maps = host_prep(x, Wq, bq, Wkv, bkv, Wo, bo)
    nc = get_nc()
    res = run_bass_kernel_spmd(nc, in_maps, list(range(NCORES)), trace=trace)
    out = np.concatenate([r["out"] for r in res.results], axis=0)
    kernel.last_results = res
    return out.astype(np.float32)
